# revision 16
# baseline (speedup 1.0000x reference)
"""BASE-layer MoE gate (balanced greedy assignment) on 8 Trainium2 cores.

Contract: kernel(**inputs) takes FULL inputs (features [8192,2048] f32,
W [16,2048] f32, b [16] f32, is_training scalar) and returns the FULL
outputs (sort_by_expert [8192] int32, gathered_scores [8192,1] f32),
matching reference.reference().

Strategy (data-parallel + replicated solve):
  - shard tokens 8 ways; each core computes its affinity shard
    aff = feat_shard @ W.T + b on the PE (fp32), plus a "packed key"
    copy of aff whose low 4 mantissa bits encode the expert id so that
    argmax ties are impossible by construction (perturbs decisions by
    <= 15 ulp, far below observed decision margins ~1e-3).
  - AllGather shards -> every core holds all 8192x16 scores+keys.
  - The reference's sequential greedy balanced assignment is the unique
    fixed point of:  a_i = argmax_e { s_ie : #(j<i, a_j=e) < 512 },
    iterated synchronously (deferred-acceptance).
  - Capacity only binds near the end for balanced random scores: if no
    expert reaches 512 among the first 6144 tokens under unconstrained
    argmax (checked on device; ~6.7 sigma), tokens < 6144 keep their
    unconstrained argmax and only the last 2048 tokens need iterating.
    Those tail rounds run on compact [128, 16*16] tiles (4x cheaper).
    A flag output reports the (astronomically unlikely) unsafe case and
    the host falls back to a full-width-rounds build of the module.
  - Final pass decodes expert ids, ranks, softmax prob of the assigned
    expert; sort_by_expert is the scatter out[pos_i] = i.

Token layout on chip: token i lives at (partition p = i % 128,
block b = i // 128); score/key tiles are [128, 16 experts, 64 blocks]
(e-major free dim so the per-partition scan runs along b within e).
"""

import numpy as np
from contextlib import ExitStack

T = 8192
D = 2048
E = 16
CAP = T // E          # 512
NCORES = 8
TSH = T // NCORES     # 1024 tokens per core
P = 128
B = T // P            # 64 blocks
KCH = D // P          # 16 contraction chunks
TT = TSH // P         # 8 token tiles per core
BCUT = 48             # tail starts at token BCUT*128 = 6144
TB = B - BCUT         # 16 tail blocks (2048 tokens)
ROUNDS = 12           # full-width rounds (fallback build)
TROUNDS = 13          # tail rounds (worst seen 10)
BIG = 1.0e30
DEVICE_SCATTER = False   # device-side indirect-DMA scatter for out_sort
DIAG = True              # extra outputs to validate scatter primitives

_CACHE = {}


def _build(tail=True):
    import concourse.bass as bass
    import concourse.bacc as bacc
    import concourse.mybir as mybir
    import concourse.tile as tile

    f32 = mybir.dt.float32
    i32 = mybir.dt.int32
    Alu = mybir.AluOpType
    Act = mybir.ActivationFunctionType
    exp_fn = getattr(Act, "Exp", None) or getattr(Act, "exp")

    nc = bacc.Bacc("TRN2", target_bir_lowering=False, debug=False,
                   num_devices=NCORES)

    feat = nc.dram_tensor("feat", [TSH, D], f32, kind="ExternalInput").ap()
    wt = nc.dram_tensor("wt", [D, E], f32, kind="ExternalInput").ap()
    bias = nc.dram_tensor("bias", [1, E], f32, kind="ExternalInput").ap()
    out_sort = nc.dram_tensor("out_sort", [T, 1], i32, kind="ExternalOutput").ap()
    out_gs = nc.dram_tensor("out_gs", [T, 1], f32, kind="ExternalOutput").ap()
    out_pos = nc.dram_tensor("out_pos", [T, 1], f32, kind="ExternalOutput").ap()
    out_flag = nc.dram_tensor("out_flag", [1, 1], f32, kind="ExternalOutput").ap()
    if DIAG:
        out_prow = nc.dram_tensor("out_prow", [1, T], i32,
                                  kind="ExternalOutput").ap()
        out_irow = nc.dram_tensor("out_irow", [1, T], i32,
                                  kind="ExternalOutput").ap()

    with tile.TileContext(nc) as tc, ExitStack() as ctx:
        persist = ctx.enter_context(tc.tile_pool(name="persist", bufs=1))
        dram = ctx.enter_context(tc.tile_pool(name="dram", bufs=1, space="DRAM"))

        # ---- constants --------------------------------------------------
        jmp = persist.tile([P, P], i32, tag="jmp")
        nc.gpsimd.iota(jmp[:], pattern=[[1, P]], base=0, channel_multiplier=-1)
        ident = persist.tile([P, P], f32, tag="ident")
        nc.vector.tensor_scalar(ident[:], jmp[:], 0, None, Alu.is_equal)
        ltones = persist.tile([P, P], f32, tag="ltones")
        nc.vector.tensor_scalar(ltones[:], jmp[:], 0, None, Alu.is_gt)
        onesf = persist.tile([P, P], f32, tag="onesf")
        nc.vector.memset(onesf[:], 1.0)
        ones1 = persist.tile([1, P], f32, tag="ones1")
        nc.vector.memset(ones1[:], 1.0)
        irev16 = persist.tile([P, E], i32, tag="irev16")
        nc.gpsimd.iota(irev16[:], pattern=[[-1, E]], base=E - 1,
                       channel_multiplier=0)

        wt_sb = persist.tile([P, KCH * E], f32, tag="wt_sb")
        nc.sync.dma_start(out=wt_sb[:].rearrange("p (k e) -> p k e", k=KCH),
                          in_=wt.rearrange("(k p) e -> p k e", p=P))
        bias_sb = persist.tile([1, E], f32, tag="bias_sb")
        nc.sync.dma_start(out=bias_sb[:], in_=bias)

        # ---- stage 1: aff = feat @ W.T + b, plus packed keys ------------
        bounce = dram.tile([TSH, 2 * E], f32, tag="bounce")
        gathered = dram.tile([T, 2 * E], f32, tag="gathered")

        with tc.tile_pool(name="s1", bufs=3) as s1, \
             tc.tile_pool(name="s1p", bufs=4, space="PSUM") as s1p, \
             tc.tile_pool(name="s1b", bufs=2) as s1b:
            for t in range(TT):
                ft = s1.tile([P, D], f32, tag="ft")
                nc.sync.dma_start(out=ft[:], in_=feat[t * P:(t + 1) * P, :])
                affp = s1p.tile([P, E], f32, tag="affp")
                for k in range(KCH):
                    tp = s1p.tile([P, P], f32, tag="tp")
                    nc.tensor.transpose(out=tp[:], in_=ft[:, k * P:(k + 1) * P],
                                        identity=ident[:])
                    fT = s1.tile([P, P], f32, tag="fT")
                    if k % 2 == 0:
                        nc.vector.tensor_copy(fT[:], tp[:])
                    else:
                        nc.scalar.copy(fT[:], tp[:])
                    nc.tensor.matmul(out=affp[:], lhsT=fT[:],
                                     rhs=wt_sb[:, k * E:(k + 1) * E],
                                     start=(k == 0), stop=False)
                nc.tensor.matmul(out=affp[:], lhsT=ones1[:], rhs=bias_sb[:],
                                 start=False, stop=True)
                ab = s1b.tile([P, 2 * E], f32, tag="ab")
                nc.vector.tensor_copy(ab[:, 0:E], affp[:])
                # packed key: (aff & ~15) | (irev16 ^ ((aff < 0) * 15))
                abi = ab[:].bitcast(i32)
                smask = s1b.tile([P, E], i32, tag="smask")
                nc.vector.tensor_scalar(smask[:], ab[:, 0:E], 0.0, 15,
                                        Alu.is_lt, Alu.mult)
                nc.vector.tensor_tensor(out=smask[:], in0=smask[:],
                                        in1=irev16[:], op=Alu.bitwise_xor)
                nc.vector.tensor_scalar(abi[:, E:2 * E], abi[:, 0:E], -16, None,
                                        Alu.bitwise_and)
                nc.vector.tensor_tensor(out=abi[:, E:2 * E], in0=abi[:, E:2 * E],
                                        in1=smask[:], op=Alu.bitwise_or)
                nc.sync.dma_start(out=bounce[t * P:(t + 1) * P, :], in_=ab[:])

        nc.gpsimd.collective_compute(
            "AllGather", mybir.AluOpType.bypass,
            replica_groups=[list(range(NCORES))],
            ins=[bounce[:].opt()], outs=[gathered[:].opt()])

        # ---- load all scores/keys, b-major -> e-major -------------------
        sck = persist.tile([P, B * 2 * E], f32, tag="sck")
        nc.sync.dma_start(out=sck[:].rearrange("p (b e) -> p b e", b=B),
                          in_=gathered[:].rearrange("(b p) e -> p b e", p=P))
        sck3 = sck[:].rearrange("p (b e) -> p b e", b=B)       # [P, B, 2E]
        S3 = persist.tile([P, E, B], f32, tag="S3")
        nc.vector.tensor_copy(S3[:], sck3[:, :, 0:E].transpose([0, 2, 1]))
        K2 = persist.tile([P, E * B], f32, tag="K2")
        K3 = K2[:].rearrange("p (e b) -> p e b", e=E)
        nc.vector.tensor_copy(K3, sck3[:, :, E:2 * E].transpose([0, 2, 1]))

        # ---- fixed point ------------------------------------------------
        oh = persist.tile([P, E * B + 1], f32, tag="oh")       # guard col 0
        nc.vector.memset(oh[:, 0:1], 0.0)
        oh3 = oh[:, 1:].rearrange("p (e b) -> p e b", e=E)
        pen = persist.tile([P, E * B], f32, tag="pen")
        pen3 = pen[:].rearrange("p (e b) -> p e b", e=E)
        rmax = persist.tile([P, 1, B], f32, tag="rmax")
        rmax2 = rmax[:, 0, :]

        rp = ctx.enter_context(tc.tile_pool(name="rp", bufs=2))
        rpp = ctx.enter_context(tc.tile_pool(name="rpp", bufs=1, space="PSUM"))
        half = E * B // 2

        def full_counts(src_oh):
            """scan + 2x2 matmuls -> exps[p,(e,b)] = ex + C[e] in PSUM."""
            zx = rp.tile([P, E * B + 1], f32, tag="zx")
            nc.vector.tensor_tensor_scan(out=zx[:], data0=src_oh[:],
                                         data1=src_oh[:], initial=0.0,
                                         op0=Alu.add, op1=Alu.bypass)
            ex = rpp.tile([P, E * B], f32, tag="exps")
            for h in range(2):
                sl = slice(h * half, (h + 1) * half)
                nc.tensor.matmul(out=ex[:, sl], lhsT=onesf[:], rhs=zx[:, sl],
                                 start=True, stop=False)
                nc.tensor.matmul(out=ex[:, sl], lhsT=ltones[:],
                                 rhs=src_oh[:, 1 + h * half:1 + (h + 1) * half],
                                 start=False, stop=True)
            return ex

        def thr_from(ex, extra, tag):
            """materialize thr[p,e] = C[e] + extra over all partitions."""
            cvec = rp.tile([1, E, 1], f32, tag="cvec" + tag)
            nc.vector.tensor_copy(
                cvec[:], ex[0:1, :].rearrange("p (e b) -> p e b", e=E)[:, :, 0:1])
            cv2 = rp.tile([1, E], f32, tag="cv2" + tag)
            if isinstance(extra, float):
                nc.vector.tensor_scalar(cv2[:], cvec[:, :, 0], extra, None,
                                        Alu.add)
            else:
                nc.vector.tensor_tensor(out=cv2[:], in0=cvec[:, :, 0],
                                        in1=extra, op=Alu.add)
            thrp = rpp.tile([P, E], f32, tag="thrp" + tag)
            nc.tensor.matmul(out=thrp[:], lhsT=ones1[:], rhs=cv2[:],
                             start=True, stop=True)
            thr3 = rp.tile([P, E, 1], f32, tag="thr3" + tag)
            nc.vector.tensor_copy(thr3[:, :, 0], thrp[:])
            return cvec, thr3

        def argmax_step(msrc, oh_dst3, rmax_dst):
            nc.vector.tensor_reduce(rmax_dst[:, 0, :],
                                    msrc.transpose([0, 2, 1]),
                                    axis=mybir.AxisListType.X, op=Alu.max)
            nc.vector.tensor_tensor(out=oh_dst3, in0=msrc,
                                    in1=rmax_dst[:].to_broadcast(
                                        [P, msrc.shape[1], msrc.shape[2]]),
                                    op=Alu.is_equal)

        # round 0: unconstrained argmax over the full width
        argmax_step(K3, oh3, rmax)
        exps = full_counts(oh)

        if not tail:
            cvec0, thr3 = thr_from(exps, 511.5, "f")
            nc.vector.tensor_tensor(out=pen3, in0=exps[:].rearrange(
                "p (e b) -> p e b", e=E),
                in1=thr3[:].to_broadcast([P, E, B]), op=Alu.is_gt)
            for r in range(1, ROUNDS + 1):
                mst = rp.tile([P, E * B], f32, tag="ms")
                nc.vector.scalar_tensor_tensor(
                    out=mst[:], in0=pen[:], scalar=-BIG, in1=K2[:],
                    op0=Alu.mult, op1=Alu.add)
                argmax_step(mst[:].rearrange("p (e b) -> p e b", e=E), oh3, rmax)
                exps = full_counts(oh)
                if r == ROUNDS:
                    break
                _, thr3 = thr_from(exps, 511.5, "f")
                nc.vector.tensor_tensor(out=pen3, in0=exps[:].rearrange(
                    "p (e b) -> p e b", e=E),
                    in1=thr3[:].to_broadcast([P, E, B]), op=Alu.is_gt)
            # flag output (always safe in this build)
            zf = rp.tile([1, 1], f32, tag="zf")
            nc.vector.memset(zf[:], 0.0)
            nc.sync.dma_start(out=out_flag, in_=zf[:])
            thr_cvec, thr_last = thr_from(exps, 511.5, "l")
        else:
            cvec0, thr3 = thr_from(exps, 511.5, "f")
            nc.vector.tensor_tensor(out=pen3, in0=exps[:].rearrange(
                "p (e b) -> p e b", e=E),
                in1=thr3[:].to_broadcast([P, E, B]), op=Alu.is_gt)
            # base_e = unconstrained count among tokens < 6144; flag if >= 512
            base3 = rp.tile([1, E, 1], f32, tag="base3")
            nc.vector.tensor_tensor(
                out=base3[:],
                in0=exps[0:1, :].rearrange("p (e b) -> p e b",
                                           e=E)[:, :, BCUT:BCUT + 1],
                in1=cvec0[:], op=Alu.subtract)
            thrbase = rp.tile([1, E], f32, tag="thrbase")
            nc.vector.tensor_scalar(thrbase[:], base3[:, :, 0], -1.0, 511.5,
                                    Alu.mult, Alu.add)
            maxb = rp.tile([1, 1], f32, tag="maxb")
            nc.vector.tensor_reduce(maxb[:], base3[:, :, 0],
                                    axis=mybir.AxisListType.X, op=Alu.max)
            nc.sync.dma_start(out=out_flag, in_=maxb[:])

            # compact tail tiles (last TB blocks of every expert)
            TBF = E * TB
            kt = persist.tile([P, TBF], f32, tag="kt")
            kt3 = kt[:].rearrange("p (e b) -> p e b", e=E)
            nc.vector.tensor_copy(kt3, K3[:, :, BCUT:])
            pent = persist.tile([P, TBF], f32, tag="pent")
            pent3 = pent[:].rearrange("p (e b) -> p e b", e=E)
            nc.vector.tensor_copy(pent3, pen3[:, :, BCUT:])
            oht = persist.tile([P, TBF + 1], f32, tag="oht")
            nc.vector.memset(oht[:, 0:1], 0.0)
            oht3 = oht[:, 1:].rearrange("p (e b) -> p e b", e=E)
            rmaxt = persist.tile([P, 1, TB], f32, tag="rmaxt")

            for r in range(1, TROUNDS + 1):
                mst = rp.tile([P, TBF], f32, tag="mst")
                nc.vector.scalar_tensor_tensor(
                    out=mst[:], in0=pent[:], scalar=-BIG, in1=kt[:],
                    op0=Alu.mult, op1=Alu.add)
                argmax_step(mst[:].rearrange("p (e b) -> p e b", e=E),
                            oht3, rmaxt)
                zxt = rp.tile([P, TBF + 1], f32, tag="zxt")
                nc.vector.tensor_tensor_scan(out=zxt[:], data0=oht[:],
                                             data1=oht[:], initial=0.0,
                                             op0=Alu.add, op1=Alu.bypass)
                expst = rpp.tile([P, TBF], f32, tag="expst")
                nc.tensor.matmul(out=expst[:], lhsT=onesf[:], rhs=zxt[:, :TBF],
                                 start=True, stop=False)
                nc.tensor.matmul(out=expst[:], lhsT=ltones[:], rhs=oht[:, 1:],
                                 start=False, stop=True)
                if r == TROUNDS:
                    break
                cvect = rp.tile([1, E, 1], f32, tag="cvect")
                nc.vector.tensor_copy(
                    cvect[:], expst[0:1, :].rearrange("p (e b) -> p e b",
                                                      e=E)[:, :, 0:1])
                cv2t = rp.tile([1, E], f32, tag="cv2t")
                nc.vector.tensor_tensor(out=cv2t[:], in0=cvect[:, :, 0],
                                        in1=thrbase[:], op=Alu.add)
                thrpt = rpp.tile([P, E], f32, tag="thrpt")
                nc.tensor.matmul(out=thrpt[:], lhsT=ones1[:], rhs=cv2t[:],
                                 start=True, stop=True)
                thr3t = rp.tile([P, E, 1], f32, tag="thr3t")
                nc.vector.tensor_copy(thr3t[:, :, 0], thrpt[:])
                nc.vector.tensor_tensor(
                    out=pent3,
                    in0=expst[:].rearrange("p (e b) -> p e b", e=E),
                    in1=thr3t[:].to_broadcast([P, E, TB]), op=Alu.is_gt)

            # merge tail results into the full-width tiles
            nc.vector.tensor_copy(oh3[:, :, BCUT:], oht3)
            nc.vector.tensor_copy(rmax[:, :, BCUT:], rmaxt[:])
            # final counts over the merged assignment
            exps = full_counts(oh)
            thr_cvec, thr_last = thr_from(exps, 511.5, "l")

        # ---- outputs ----------------------------------------------------
        fin = ctx.enter_context(tc.tile_pool(name="fin", bufs=1))
        finp = ctx.enter_context(tc.tile_pool(name="finp", bufs=1, space="PSUM"))

        # expert id from packed rmax nibble: pos: e = 15-nib ; neg: e = nib
        rmi = rmax2.bitcast(i32)
        nib = fin.tile([P, B], i32, tag="nib")
        nc.vector.tensor_scalar(nib[:], rmi, 15, None, Alu.bitwise_and)
        sgn = fin.tile([P, B], i32, tag="sgn")
        nc.vector.tensor_scalar(sgn[:], rmax2, 0.0, None, Alu.is_lt)
        t2 = fin.tile([P, B], i32, tag="t2")
        nc.vector.tensor_scalar(t2[:], nib[:], 2, 15, Alu.mult, Alu.subtract)
        nc.vector.tensor_tensor(out=t2[:], in0=t2[:], in1=sgn[:], op=Alu.mult)
        aexp = fin.tile([P, B], i32, tag="aexp")
        nc.vector.tensor_scalar(aexp[:], nib[:], -1, 15, Alu.mult, Alu.add)
        nc.vector.tensor_tensor(out=aexp[:], in0=aexp[:], in1=t2[:], op=Alu.add)
        aexpf = fin.tile([P, B], f32, tag="aexpf")
        nc.vector.tensor_copy(aexpf[:], aexp[:])

        # rank of each token within its expert: sum_e oh * (exps - C[e])
        cmat = fin.tile([P, E, 1], f32, tag="cmat")
        nc.vector.tensor_scalar(cmat[:, :, 0], thr_last[:, :, 0], 511.5, None,
                                Alu.subtract)
        exc = fin.tile([P, E, B], f32, tag="exc")
        nc.vector.tensor_tensor(out=exc[:],
                                in0=exps[:].rearrange("p (e b) -> p e b", e=E),
                                in1=cmat[:].to_broadcast([P, E, B]),
                                op=Alu.subtract)
        nc.vector.tensor_tensor(out=exc[:], in0=exc[:], in1=oh3, op=Alu.mult)
        rank = fin.tile([P, 1, B], f32, tag="rank")
        nc.vector.tensor_reduce(rank[:, 0, :], exc[:].transpose([0, 2, 1]),
                                axis=mybir.AxisListType.X, op=Alu.add)
        posf = fin.tile([P, B], f32, tag="posf")
        nc.vector.scalar_tensor_tensor(out=posf[:], in0=aexpf[:],
                                       scalar=float(CAP), in1=rank[:, 0, :],
                                       op0=Alu.mult, op1=Alu.add)
        posi = fin.tile([P, B], i32, tag="posi")
        nc.vector.tensor_copy(posi[:], posf[:])

        # gathered score: softmax prob of assigned expert
        dd = fin.tile([P, E, B], f32, tag="dd")
        nc.vector.tensor_tensor(out=dd[:], in0=S3[:],
                                in1=rmax[:].to_broadcast([P, E, B]),
                                op=Alu.subtract)
        ed = fin.tile([P, E, B], f32, tag="ed")
        nc.scalar.activation(ed[:], dd[:], exp_fn)
        zs = fin.tile([P, 1, B], f32, tag="zs")
        nc.vector.tensor_reduce(zs[:, 0, :], ed[:].transpose([0, 2, 1]),
                                axis=mybir.AxisListType.X, op=Alu.add)
        nc.vector.tensor_tensor(out=ed[:], in0=ed[:], in1=oh3, op=Alu.mult)
        numr = fin.tile([P, 1, B], f32, tag="numr")
        nc.vector.tensor_reduce(numr[:, 0, :], ed[:].transpose([0, 2, 1]),
                                axis=mybir.AxisListType.X, op=Alu.add)
        rz = fin.tile([P, B], f32, tag="rz")
        nc.vector.reciprocal(rz[:], zs[:, 0, :])
        gsc = fin.tile([P, B], f32, tag="gsc")
        nc.vector.tensor_tensor(out=gsc[:], in0=numr[:, 0, :], in1=rz[:],
                                op=Alu.mult)

        # transpose [P, B] -> [B, P] so output rows are contiguous
        gtp = finp.tile([B, P], f32, tag="gtp")
        nc.tensor.transpose(out=gtp[:], in_=gsc[:], identity=ident[:])
        gts = fin.tile([B, P], f32, tag="gts")
        nc.vector.tensor_copy(gts[:], gtp[:])
        nc.sync.dma_start(out=out_gs.rearrange("(b p) x -> b p x", p=P),
                          in_=gts[:].rearrange("b (p x) -> b p x", x=1))

        ptp = finp.tile([B, P], f32, tag="ptp")
        nc.tensor.transpose(out=ptp[:], in_=posf[:], identity=ident[:])
        pts = fin.tile([B, P], f32, tag="pts")
        nc.vector.tensor_copy(pts[:], ptp[:])
        nc.sync.dma_start(out=out_pos.rearrange("(b p) x -> b p x", p=P),
                          in_=pts[:].rearrange("b (p x) -> b p x", x=1))

        # scatter machinery (single-partition-row offsets + data)
        posrow = fin.tile([1, T], i32, tag="posrow")
        nc.sync.dma_start(
            out=posrow[0:1, :].rearrange("x (p b) -> x p b", p=P),
            in_=posi[:, :])
        iotarow = fin.tile([1, T], i32, tag="iotarow")
        nc.gpsimd.iota(iotarow[0:1, :].rearrange("x (p b) -> x p b", p=P),
                       pattern=[[1, P], [P, B]], base=0, channel_multiplier=0)
        if DIAG:
            nc.sync.dma_start(out=out_prow, in_=posrow[:])
            nc.sync.dma_start(out=out_irow, in_=iotarow[:])
        if DEVICE_SCATTER:
            nc.gpsimd.indirect_dma_start(
                out=out_sort[:, :],
                out_offset=bass.IndirectOffsetOnAxis(ap=posrow[0:1, :], axis=0),
                in_=iotarow[0:1, :], in_offset=None,
                bounds_check=T - 1, oob_is_err=False)
        else:
            # out_sort unused in this configuration (host applies the
            # permutation from out_pos); write pos so the tensor is bound.
            nc.sync.dma_start(
                out=out_sort.rearrange("(b p) x -> p b x", p=P),
                in_=posi[:, :].rearrange("p (b x) -> p b x", x=1))

    nc.compile()
    return nc


def _get_nc(tail=True):
    key = f"nc_tail{tail}"
    if key not in _CACHE:
        _CACHE[key] = _build(tail=tail)
    return _CACHE[key]


def _run(nc, features, W, b):
    from concourse.bass_utils import run_bass_kernel_spmd

    f = np.ascontiguousarray(np.asarray(features, dtype=np.float32))
    wt = np.ascontiguousarray(np.asarray(W, dtype=np.float32).T)
    bb = np.ascontiguousarray(np.asarray(b, dtype=np.float32).reshape(1, E))
    in_maps = [
        {"feat": f[c * TSH:(c + 1) * TSH], "wt": wt, "bias": bb}
        for c in range(NCORES)
    ]
    return run_bass_kernel_spmd(nc, in_maps, list(range(NCORES)))


def _finish(r0):
    _CACHE["last_pos"] = np.asarray(r0["out_pos"], dtype=np.float32).reshape(T)
    _CACHE["last_diag"] = {k: np.asarray(r0[k]) for k in ("out_prow", "out_irow")
                           if k in r0}
    if DEVICE_SCATTER:
        sort = np.asarray(r0["out_sort"], dtype=np.int32).reshape(T)
    else:
        pos = _CACHE["last_pos"].astype(np.int64)
        sort = np.zeros(T, dtype=np.int32)
        sort[pos] = np.arange(T, dtype=np.int32)
    return (sort, np.asarray(r0["out_gs"], dtype=np.float32).reshape(T, 1))


def kernel(features, W, b, is_training=None, **kw):
    res = _run(_get_nc(tail=True), features, W, b)
    r0 = res.results[0]
    flag = float(np.asarray(r0["out_flag"]).reshape(-1)[0])
    if flag >= CAP - 0.5:
        # pathological distribution: an expert fills before token 6144.
        # Rebuild with full-width rounds (exact for any input).
        res = _run(_get_nc(tail=False), features, W, b)
        r0 = res.results[0]
    return _finish(r0)


# revision 20
# speedup vs baseline: 1.3701x; 1.3701x over previous
"""BASE-layer MoE gate (balanced greedy assignment) on 8 Trainium2 cores.

Contract: kernel(**inputs) takes FULL inputs (features [8192,2048] f32,
W [16,2048] f32, b [16] f32, is_training scalar) and returns the FULL
outputs (sort_by_expert [8192] int32, gathered_scores [8192,1] f32),
matching reference.reference().

Strategy (data-parallel + replicated solve):
  - shard tokens 8 ways; each core computes its affinity shard
    aff = feat_shard @ W.T + b on the PE (fp32), plus a "packed key"
    copy of aff whose low 4 mantissa bits encode the expert id so that
    argmax ties are impossible by construction (perturbs decisions by
    <= 15 ulp, far below observed decision margins ~1e-3).
  - AllGather shards -> every core holds all 8192x16 scores+keys.
  - The reference's sequential greedy balanced assignment is the unique
    fixed point of:  a_i = argmax_e { s_ie : #(j<i, a_j=e) < 512 },
    iterated synchronously (deferred-acceptance).
  - Capacity only binds near the end for balanced random scores: if no
    expert reaches 512 among the first 6144 tokens under unconstrained
    argmax (checked on device; ~6.7 sigma), tokens < 6144 keep their
    unconstrained argmax and only the last 2048 tokens need iterating.
    Those tail rounds run on compact [128, 16*16] tiles (4x cheaper).
    A flag output reports the (astronomically unlikely) unsafe case and
    the host falls back to a full-width-rounds build of the module.
  - Final pass decodes expert ids, ranks, softmax prob of the assigned
    expert; sort_by_expert is the scatter out[pos_i] = i.

Token layout on chip: token i lives at (partition p = i % 128,
block b = i // 128); score/key tiles are [128, 16 experts, 64 blocks]
(e-major free dim so the per-partition scan runs along b within e).
"""

import numpy as np
from contextlib import ExitStack

T = 8192
D = 2048
E = 16
CAP = T // E          # 512
NCORES = 8
TSH = T // NCORES     # 1024 tokens per core
P = 128
B = T // P            # 64 blocks
KCH = D // P          # 16 contraction chunks
TT = TSH // P         # 8 token tiles per core
BCUT = 48             # tail starts at token BCUT*128 = 6144
TB = B - BCUT         # 16 tail blocks (2048 tokens)
ROUNDS = 12           # full-width rounds (fallback build)
TROUNDS = 13          # tail rounds (worst seen 10)
BIG = 1.0e30
DEVICE_SCATTER = False   # device-side indirect-DMA scatter for out_sort
DIAG = True              # extra outputs to validate scatter primitives

_CACHE = {}


def _build(tail=True):
    import concourse.bass as bass
    import concourse.bacc as bacc
    import concourse.mybir as mybir
    import concourse.tile as tile

    f32 = mybir.dt.float32
    f16 = mybir.dt.float16
    i32 = mybir.dt.int32
    Alu = mybir.AluOpType
    Act = mybir.ActivationFunctionType
    exp_fn = getattr(Act, "Exp", None) or getattr(Act, "exp")

    nc = bacc.Bacc("TRN2", target_bir_lowering=False, debug=False,
                   num_devices=NCORES)

    feat = nc.dram_tensor("feat", [TSH, D], f32, kind="ExternalInput").ap()
    wt = nc.dram_tensor("wt", [D, E], f32, kind="ExternalInput").ap()
    bias = nc.dram_tensor("bias", [1, E], f32, kind="ExternalInput").ap()
    out_sort = nc.dram_tensor("out_sort", [T, 1], i32, kind="ExternalOutput").ap()
    out_gs = nc.dram_tensor("out_gs", [T, 1], f32, kind="ExternalOutput").ap()
    out_pos = nc.dram_tensor("out_pos", [T, 1], f32, kind="ExternalOutput").ap()
    out_flag = nc.dram_tensor("out_flag", [1, 1], f32, kind="ExternalOutput").ap()

    with tile.TileContext(nc) as tc, ExitStack() as ctx:
        persist = ctx.enter_context(tc.tile_pool(name="persist", bufs=1))
        dram = ctx.enter_context(tc.tile_pool(name="dram", bufs=1, space="DRAM"))

        # ---- constants --------------------------------------------------
        jmp = persist.tile([P, P], i32, tag="jmp")
        nc.gpsimd.iota(jmp[:], pattern=[[1, P]], base=0, channel_multiplier=-1)
        ident = persist.tile([P, P], f32, tag="ident")
        nc.vector.tensor_scalar(ident[:], jmp[:], 0, None, Alu.is_equal)
        ltones = persist.tile([P, P], f32, tag="ltones")
        nc.vector.tensor_scalar(ltones[:], jmp[:], 0, None, Alu.is_gt)
        onesf = persist.tile([P, P], f32, tag="onesf")
        nc.vector.memset(onesf[:], 1.0)
        onesh = persist.tile([P, P], f16, tag="onesh")
        nc.vector.memset(onesh[:], 1.0)
        ltonesh = persist.tile([P, P], f16, tag="ltonesh")
        nc.vector.tensor_scalar(ltonesh[:], jmp[:], 0, None, Alu.is_gt)
        ones1 = persist.tile([1, P], f32, tag="ones1")
        nc.vector.memset(ones1[:], 1.0)
        irev16 = persist.tile([P, E], i32, tag="irev16")
        nc.gpsimd.iota(irev16[:], pattern=[[-1, E]], base=E - 1,
                       channel_multiplier=0)

        wt_sb = persist.tile([P, KCH * E], f32, tag="wt_sb")
        nc.sync.dma_start(out=wt_sb[:].rearrange("p (k e) -> p k e", k=KCH),
                          in_=wt.rearrange("(k p) e -> p k e", p=P))
        bias_sb = persist.tile([1, E], f32, tag="bias_sb")
        nc.sync.dma_start(out=bias_sb[:], in_=bias)

        # ---- stage 1: aff = feat @ W.T + b, plus packed keys ------------
        HT = TT // 2                                   # tiles per piece
        bounce = [dram.tile([HT * P, 2 * E], f32, tag=f"bounce{j}",
                             name=f"bounce{j}")
                  for j in range(2)]
        gath = [dram.tile([NCORES * HT * P, 2 * E], f32, tag=f"gath{j}",
                           name=f"gath{j}")
                for j in range(2)]
        sckg = persist.tile([P, B * 2 * E], f32, tag="sckg")

        with tc.tile_pool(name="s1", bufs=3) as s1, \
             tc.tile_pool(name="s1p", bufs=4, space="PSUM") as s1p, \
             tc.tile_pool(name="s1b", bufs=2) as s1b:
            for t in range(TT):
                ft = s1.tile([P, D], f32, tag="ft")
                nc.sync.dma_start(out=ft[:], in_=feat[t * P:(t + 1) * P, :])
                affp = s1p.tile([P, E], f32, tag="affp")
                fTs = []
                for k in range(KCH):
                    tp = s1p.tile([P, P], f32, tag="tp")
                    nc.tensor.transpose(out=tp[:], in_=ft[:, k * P:(k + 1) * P],
                                        identity=ident[:])
                    fT = s1.tile([P, P], f32, tag=f"fT{k % 4}")
                    if k % 2 == 0:
                        nc.vector.tensor_copy(fT[:], tp[:])
                    else:
                        nc.scalar.copy(fT[:], tp[:])
                    fTs.append(fT)
                for k in range(KCH):
                    nc.tensor.matmul(out=affp[:], lhsT=fTs[k][:],
                                     rhs=wt_sb[:, k * E:(k + 1) * E],
                                     start=(k == 0), stop=False)
                nc.tensor.matmul(out=affp[:], lhsT=ones1[:], rhs=bias_sb[:],
                                 start=False, stop=True)
                ab_t = s1b.tile([P, 2 * E], f32, tag="ab")
                ab = ab_t[:]
                nc.vector.tensor_copy(ab[:, 0:E], affp[:])
                # packed key: (aff & ~15) | (irev16 ^ ((aff < 0) * 15))
                abi = ab.bitcast(i32)
                smask = s1b.tile([P, E], i32, tag="smask")
                nc.vector.tensor_scalar(smask[:], ab[:, 0:E], 0.0, 15,
                                        Alu.is_lt, Alu.mult)
                nc.vector.tensor_tensor(out=smask[:], in0=smask[:],
                                        in1=irev16[:], op=Alu.bitwise_xor)
                nc.vector.tensor_scalar(abi[:, E:2 * E], abi[:, 0:E], -16, None,
                                        Alu.bitwise_and)
                nc.vector.tensor_tensor(out=abi[:, E:2 * E], in0=abi[:, E:2 * E],
                                        in1=smask[:], op=Alu.bitwise_or)
                j, u = divmod(t, HT)
                nc.sync.dma_start(out=bounce[j][u * P:(u + 1) * P, :], in_=ab)
                if u == HT - 1:
                    # gather this half while the next half still computes
                    nc.gpsimd.collective_compute(
                        "AllGather", mybir.AluOpType.bypass,
                        replica_groups=[list(range(NCORES))],
                        ins=[bounce[j][:].opt()], outs=[gath[j][:].opt()])

        # gathered piece j rows: c*HT*P + u*P + p  <->  token (c*8+j*HT+u)*P+p
        sckg4 = sckg[:].rearrange("p (c u e) -> p c u e", c=NCORES, u=TT)
        for j in range(2):
            g4 = gath[j][:].rearrange("(c u p) e -> c p u e", c=NCORES, u=HT)
            for c in range(NCORES):
                nc.sync.dma_start(
                    out=sckg4[:, c, j * HT:(j + 1) * HT, :],
                    in_=g4[c])

        # ---- all scores/keys, b-major -> e-major ------------------------
        sck3 = sckg[:].rearrange("p (b e) -> p b e", b=B)      # [P, B, 2E]
        S3 = persist.tile([P, E, B], f32, tag="S3")
        nc.vector.tensor_copy(S3[:], sck3[:, :, 0:E].transpose([0, 2, 1]))
        K2 = persist.tile([P, E * B], f32, tag="K2")
        K3 = K2[:].rearrange("p (e b) -> p e b", e=E)
        nc.vector.tensor_copy(K3, sck3[:, :, E:2 * E].transpose([0, 2, 1]))

        # ---- fixed point ------------------------------------------------
        oh = persist.tile([P, E * B + 1], f32, tag="oh")       # guard col 0
        nc.vector.memset(oh[:, 0:1], 0.0)
        oh3 = oh[:, 1:].rearrange("p (e b) -> p e b", e=E)
        pen = persist.tile([P, E * B], f32, tag="pen")
        pen3 = pen[:].rearrange("p (e b) -> p e b", e=E)
        rmax = persist.tile([P, 1, B], f32, tag="rmax")
        rmax2 = rmax[:, 0, :]

        rp = ctx.enter_context(tc.tile_pool(name="rp", bufs=2))
        rpp = ctx.enter_context(tc.tile_pool(name="rpp", bufs=1, space="PSUM"))
        half = E * B // 2

        def full_counts(src_oh):
            """scan + 2x2 matmuls -> exps[p,(e,b)] = ex + C[e] in PSUM."""
            zx = rp.tile([P, E * B + 1], f32, tag="zx")
            nc.vector.tensor_tensor_scan(out=zx[:], data0=src_oh[:],
                                         data1=src_oh[:], initial=0.0,
                                         op0=Alu.add, op1=Alu.bypass)
            ex = rpp.tile([P, E * B], f32, tag="exps")
            for h in range(2):
                sl = slice(h * half, (h + 1) * half)
                nc.tensor.matmul(out=ex[:, sl], lhsT=onesf[:], rhs=zx[:, sl],
                                 start=True, stop=False)
                nc.tensor.matmul(out=ex[:, sl], lhsT=ltones[:],
                                 rhs=src_oh[:, 1 + h * half:1 + (h + 1) * half],
                                 start=False, stop=True)
            return ex

        def thr_from(ex, extra, tag):
            """materialize thr[p,e] = C[e] + extra over all partitions."""
            cvec = rp.tile([1, E, 1], f32, tag="cvec" + tag)
            nc.vector.tensor_copy(
                cvec[:], ex[0:1, :].rearrange("p (e b) -> p e b", e=E)[:, :, 0:1])
            cv2 = rp.tile([1, E], f32, tag="cv2" + tag)
            if isinstance(extra, float):
                nc.vector.tensor_scalar(cv2[:], cvec[:, :, 0], extra, None,
                                        Alu.add)
            else:
                nc.vector.tensor_tensor(out=cv2[:], in0=cvec[:, :, 0],
                                        in1=extra, op=Alu.add)
            thrp = rpp.tile([P, E], f32, tag="thrp" + tag)
            nc.tensor.matmul(out=thrp[:], lhsT=ones1[:], rhs=cv2[:],
                             start=True, stop=True)
            thr3 = rp.tile([P, E, 1], f32, tag="thr3" + tag)
            nc.vector.tensor_copy(thr3[:, :, 0], thrp[:])
            return cvec, thr3

        def argmax_step(msrc, oh_dst3, rmax_dst):
            nc.vector.tensor_reduce(rmax_dst[:, 0, :],
                                    msrc.transpose([0, 2, 1]),
                                    axis=mybir.AxisListType.X, op=Alu.max)
            nc.vector.tensor_tensor(out=oh_dst3, in0=msrc,
                                    in1=rmax_dst[:].to_broadcast(
                                        [P, msrc.shape[1], msrc.shape[2]]),
                                    op=Alu.is_equal)

        # round 0: unconstrained argmax over the full width
        argmax_step(K3, oh3, rmax)
        exps = full_counts(oh)

        if not tail:
            cvec0, thr3 = thr_from(exps, 511.5, "f")
            nc.vector.tensor_tensor(out=pen3, in0=exps[:].rearrange(
                "p (e b) -> p e b", e=E),
                in1=thr3[:].to_broadcast([P, E, B]), op=Alu.is_gt)
            for r in range(1, ROUNDS + 1):
                mst = rp.tile([P, E * B], f32, tag="ms")
                nc.vector.scalar_tensor_tensor(
                    out=mst[:], in0=pen[:], scalar=-BIG, in1=K2[:],
                    op0=Alu.mult, op1=Alu.add)
                argmax_step(mst[:].rearrange("p (e b) -> p e b", e=E), oh3, rmax)
                exps = full_counts(oh)
                if r == ROUNDS:
                    break
                _, thr3 = thr_from(exps, 511.5, "f")
                nc.vector.tensor_tensor(out=pen3, in0=exps[:].rearrange(
                    "p (e b) -> p e b", e=E),
                    in1=thr3[:].to_broadcast([P, E, B]), op=Alu.is_gt)
            # flag output (always safe in this build)
            zf = rp.tile([1, 1], f32, tag="zf")
            nc.vector.memset(zf[:], 0.0)
            nc.sync.dma_start(out=out_flag, in_=zf[:])
            thr_cvec, thr_last = thr_from(exps, 511.5, "l")
        else:
            cvec0, thr3 = thr_from(exps, 511.5, "f")
            # base_e = unconstrained count among tokens < 6144; flag if >= 512
            base3 = rp.tile([1, E, 1], f32, tag="base3")
            nc.vector.tensor_tensor(
                out=base3[:],
                in0=exps[0:1, :].rearrange("p (e b) -> p e b",
                                           e=E)[:, :, BCUT:BCUT + 1],
                in1=cvec0[:], op=Alu.subtract)
            thrbase = rp.tile([1, E], f32, tag="thrbase")
            nc.vector.tensor_scalar(thrbase[:], base3[:, :, 0], -1.0, 511.5,
                                    Alu.mult, Alu.add)
            maxb = rp.tile([1, 1], f32, tag="maxb")
            nc.vector.tensor_reduce(maxb[:], base3[:, :, 0],
                                    axis=mybir.AxisListType.X, op=Alu.max)
            nc.sync.dma_start(out=out_flag, in_=maxb[:])

            # compact tail tiles (last TB blocks of every expert)
            TBF = E * TB
            kt = persist.tile([P, TBF], f32, tag="kt")
            kt3 = kt[:].rearrange("p (e b) -> p e b", e=E)
            nc.vector.tensor_copy(kt3, K3[:, :, BCUT:])
            pent = persist.tile([P, TBF], f32, tag="pent")
            pent3 = pent[:].rearrange("p (e b) -> p e b", e=E)
            nc.vector.tensor_tensor(
                out=pent3,
                in0=exps[:].rearrange("p (e b) -> p e b", e=E)[:, :, BCUT:],
                in1=thr3[:].to_broadcast([P, E, TB]), op=Alu.is_gt)
            oht = persist.tile([P, TBF + 1], f16, tag="oht")
            nc.vector.memset(oht[:, 0:1], 0.0)
            oht3 = oht[:, 1:].rearrange("p (e b) -> p e b", e=E)
            rmaxt = persist.tile([P, 1, TB], f32, tag="rmaxt")

            for r in range(1, TROUNDS + 1):
                mst = rp.tile([P, TBF], f32, tag="mst")
                nc.vector.scalar_tensor_tensor(
                    out=mst[:], in0=pent[:], scalar=-BIG, in1=kt[:],
                    op0=Alu.mult, op1=Alu.add)
                argmax_step(mst[:].rearrange("p (e b) -> p e b", e=E),
                            oht3, rmaxt)
                zxt = rp.tile([P, TBF + 1], f16, tag="zxt")
                nc.vector.tensor_tensor_scan(out=zxt[:], data0=oht[:],
                                             data1=oht[:], initial=0.0,
                                             op0=Alu.add, op1=Alu.bypass)
                expst = rpp.tile([P, TBF], f32, tag="expst")
                nc.tensor.matmul(out=expst[:], lhsT=onesh[:], rhs=zxt[:, :TBF],
                                 start=True, stop=False)
                nc.tensor.matmul(out=expst[:], lhsT=ltonesh[:], rhs=oht[:, 1:],
                                 start=False, stop=True)
                if r == TROUNDS:
                    break
                cvect = rp.tile([1, E, 1], f32, tag="cvect")
                nc.vector.tensor_copy(
                    cvect[:], expst[0:1, :].rearrange("p (e b) -> p e b",
                                                      e=E)[:, :, 0:1])
                cv2t = rp.tile([1, E], f32, tag="cv2t")
                nc.vector.tensor_tensor(out=cv2t[:], in0=cvect[:, :, 0],
                                        in1=thrbase[:], op=Alu.add)
                thrpt = rpp.tile([P, E], f32, tag="thrpt")
                nc.tensor.matmul(out=thrpt[:], lhsT=ones1[:], rhs=cv2t[:],
                                 start=True, stop=True)
                thr3t = rp.tile([P, E, 1], f32, tag="thr3t")
                nc.vector.tensor_copy(thr3t[:, :, 0], thrpt[:])
                nc.vector.tensor_tensor(
                    out=pent3,
                    in0=expst[:].rearrange("p (e b) -> p e b", e=E),
                    in1=thr3t[:].to_broadcast([P, E, TB]), op=Alu.is_gt)

            # merge tail results into the full-width tiles
            nc.vector.tensor_copy(oh3[:, :, BCUT:], oht3)
            nc.vector.tensor_copy(rmax[:, :, BCUT:], rmaxt[:])
            # final counts over the merged assignment
            exps = full_counts(oh)
            thr_cvec, thr_last = thr_from(exps, 511.5, "l")

        # ---- outputs ----------------------------------------------------
        fin = ctx.enter_context(tc.tile_pool(name="fin", bufs=1))
        finp = ctx.enter_context(tc.tile_pool(name="finp", bufs=1, space="PSUM"))

        # expert id from packed rmax nibble: pos: e = 15-nib ; neg: e = nib
        rmi = rmax2.bitcast(i32)
        nib = fin.tile([P, B], i32, tag="nib")
        nc.vector.tensor_scalar(nib[:], rmi, 15, None, Alu.bitwise_and)
        sgn = fin.tile([P, B], i32, tag="sgn")
        nc.vector.tensor_scalar(sgn[:], rmax2, 0.0, None, Alu.is_lt)
        t2 = fin.tile([P, B], i32, tag="t2")
        nc.vector.tensor_scalar(t2[:], nib[:], 2, 15, Alu.mult, Alu.subtract)
        nc.vector.tensor_tensor(out=t2[:], in0=t2[:], in1=sgn[:], op=Alu.mult)
        aexp = fin.tile([P, B], i32, tag="aexp")
        nc.vector.tensor_scalar(aexp[:], nib[:], -1, 15, Alu.mult, Alu.add)
        nc.vector.tensor_tensor(out=aexp[:], in0=aexp[:], in1=t2[:], op=Alu.add)
        aexpf = fin.tile([P, B], f32, tag="aexpf")
        nc.vector.tensor_copy(aexpf[:], aexp[:])

        # rank of each token within its expert: sum_e oh * (exps - C[e])
        cmat = fin.tile([P, E, 1], f32, tag="cmat")
        nc.vector.tensor_scalar(cmat[:, :, 0], thr_last[:, :, 0], 511.5, None,
                                Alu.subtract)
        exc = fin.tile([P, E, B], f32, tag="exc")
        nc.vector.tensor_tensor(out=exc[:],
                                in0=exps[:].rearrange("p (e b) -> p e b", e=E),
                                in1=cmat[:].to_broadcast([P, E, B]),
                                op=Alu.subtract)
        nc.vector.tensor_tensor(out=exc[:], in0=exc[:], in1=oh3, op=Alu.mult)
        rank = fin.tile([P, 1, B], f32, tag="rank")
        nc.vector.tensor_reduce(rank[:, 0, :], exc[:].transpose([0, 2, 1]),
                                axis=mybir.AxisListType.X, op=Alu.add)
        posf = fin.tile([P, B], f32, tag="posf")
        nc.vector.scalar_tensor_tensor(out=posf[:], in0=aexpf[:],
                                       scalar=float(CAP), in1=rank[:, 0, :],
                                       op0=Alu.mult, op1=Alu.add)
        posi = fin.tile([P, B], i32, tag="posi")
        nc.vector.tensor_copy(posi[:], posf[:])

        # gathered score: softmax prob of assigned expert
        dd = fin.tile([P, E, B], f32, tag="dd")
        nc.vector.tensor_tensor(out=dd[:], in0=S3[:],
                                in1=rmax[:].to_broadcast([P, E, B]),
                                op=Alu.subtract)
        ed = fin.tile([P, E, B], f32, tag="ed")
        nc.scalar.activation(ed[:], dd[:], exp_fn)
        zs = fin.tile([P, 1, B], f32, tag="zs")
        nc.vector.tensor_reduce(zs[:, 0, :], ed[:].transpose([0, 2, 1]),
                                axis=mybir.AxisListType.X, op=Alu.add)
        nc.vector.tensor_tensor(out=ed[:], in0=ed[:], in1=oh3, op=Alu.mult)
        numr = fin.tile([P, 1, B], f32, tag="numr")
        nc.vector.tensor_reduce(numr[:, 0, :], ed[:].transpose([0, 2, 1]),
                                axis=mybir.AxisListType.X, op=Alu.add)
        rz = fin.tile([P, B], f32, tag="rz")
        nc.vector.reciprocal(rz[:], zs[:, 0, :])
        gsc = fin.tile([P, B], f32, tag="gsc")
        nc.vector.tensor_tensor(out=gsc[:], in0=numr[:, 0, :], in1=rz[:],
                                op=Alu.mult)

        # transpose [P, B] -> [B, P] so output rows are contiguous
        gtp = finp.tile([B, P], f32, tag="gtp")
        nc.tensor.transpose(out=gtp[:], in_=gsc[:], identity=ident[:])
        gts = fin.tile([B, P], f32, tag="gts")
        nc.vector.tensor_copy(gts[:], gtp[:])
        nc.sync.dma_start(out=out_gs.rearrange("(b p) x -> b p x", p=P),
                          in_=gts[:].rearrange("b (p x) -> b p x", x=1))

        ptp = finp.tile([B, P], f32, tag="ptp")
        nc.tensor.transpose(out=ptp[:], in_=posf[:], identity=ident[:])
        pts = fin.tile([B, P], f32, tag="pts")
        nc.vector.tensor_copy(pts[:], ptp[:])
        nc.sync.dma_start(out=out_pos.rearrange("(b p) x -> b p x", p=P),
                          in_=pts[:].rearrange("b (p x) -> b p x", x=1))

        if DEVICE_SCATTER:
            posrow = fin.tile([1, T], i32, tag="posrow")
            nc.sync.dma_start(
                out=posrow[0:1, :].rearrange("x (p b) -> x p b", p=P),
                in_=posi[:, :])
            iotarow = fin.tile([1, T], i32, tag="iotarow")
            nc.gpsimd.iota(iotarow[0:1, :].rearrange("x (p b) -> x p b", p=P),
                           pattern=[[1, P], [P, B]], base=0,
                           channel_multiplier=0)
            nc.gpsimd.indirect_dma_start(
                out=out_sort[:, :],
                out_offset=bass.IndirectOffsetOnAxis(ap=posrow[0:1, :], axis=0),
                in_=iotarow[0:1, :], in_offset=None,
                bounds_check=T - 1, oob_is_err=False)

    nc.compile()
    return nc


def _get_nc(tail=True):
    key = f"nc_tail{tail}"
    if key not in _CACHE:
        _CACHE[key] = _build(tail=tail)
    return _CACHE[key]


def _run(nc, features, W, b):
    from concourse.bass_utils import run_bass_kernel_spmd

    f = np.ascontiguousarray(np.asarray(features, dtype=np.float32))
    wt = np.ascontiguousarray(np.asarray(W, dtype=np.float32).T)
    bb = np.ascontiguousarray(np.asarray(b, dtype=np.float32).reshape(1, E))
    in_maps = [
        {"feat": f[c * TSH:(c + 1) * TSH], "wt": wt, "bias": bb}
        for c in range(NCORES)
    ]
    return run_bass_kernel_spmd(nc, in_maps, list(range(NCORES)))


def _finish(r0):
    _CACHE["last_pos"] = np.asarray(r0["out_pos"], dtype=np.float32).reshape(T)
    _CACHE["last_diag"] = {k: np.asarray(r0[k]) for k in ("out_prow", "out_irow")
                           if k in r0}
    if DEVICE_SCATTER:
        sort = np.asarray(r0["out_sort"], dtype=np.int32).reshape(T)
    else:
        pos = _CACHE["last_pos"].astype(np.int64)
        sort = np.zeros(T, dtype=np.int32)
        sort[pos] = np.arange(T, dtype=np.int32)
    return (sort, np.asarray(r0["out_gs"], dtype=np.float32).reshape(T, 1))


def kernel(features, W, b, is_training=None, **kw):
    res = _run(_get_nc(tail=True), features, W, b)
    r0 = res.results[0]
    flag = float(np.asarray(r0["out_flag"]).reshape(-1)[0])
    if flag >= CAP - 0.5:
        # pathological distribution: an expert fills before token 6144.
        # Rebuild with full-width rounds (exact for any input).
        res = _run(_get_nc(tail=False), features, W, b)
        r0 = res.results[0]
    return _finish(r0)
